# revision 16
# baseline (speedup 1.0000x reference)
"""Column-parallel linear Y = X @ W^T + b on 8 Trainium2 NeuronCores.

Strategy: sequence-shard X across the 8 cores (4096 tokens each); every core
holds the full weight, computes its token slab against all 4096 output
features, so no collective is needed and no core re-reads another's tokens.

Final version (476us vs 511us staged baseline; PE-stream roofline 437us):
  * bf16 matmul operands at the PE's 1 col/cycle: measured 216ns issue gap
    per N=512 matmul (the NX floor) vs 227ns for fp32r, whose per-matmul
    4-byte self-loading weight path leaks ~11ns into the gap. bf16 also
    halves input HBM traffic, which removes the m-tile-0 weight-starvation
    stalls. (Note: chip-wide 2.4->2.0GHz downclock episodes hit runs at
    random and masquerade as dtype effects; compare spacing, not totals.)
  * 40 full-K warmup matmuls on a memset tile run under the ~7us framework
    preamble + DMA lead-in so HAM is at K=8/8 when real work starts
    (K=1 matmuls do NOT register as PE activity for HAM).
  * bias rides in as a 16KB row and is broadcast on-chip with K=1 matmuls
    against a memset ones row (a 2.1MB DMA broadcast would fight the
    prologue for HBM bandwidth). Keep it ONE DMA: chunked bias loads from a
    small pool head-of-line block the sync queue behind PE progress.
  * xm0/w_col0 are ko-split in half so the first accumulation group only
    waits on ~1MB; W columns alternate between the sync and gpsimd rings so
    each ring sustains ~140GB/s during m-tile 0; the next m-tile prefetch
    is deferred to mid-sweep.
  * outputs are written bf16 and upcast on host: halves the 64MB of writes
    that compete with the W stream early on. Total norm rel err ~2.6e-3
    vs the 2e-2 gate.

Device layout (per core):
  xT   [128, 8, 4096]  bf16   xT[p, ko, m] = X_shard[m, ko*128 + p]
  wT   [128, 8, 4096]  bf16   wT[p, ko, n] = W[n, ko*128 + p]
  bias [4096]          fp32   (fp32r bits for the K=1 broadcast matmuls)
  out  [128, 32, 4096] bf16   out[p, mo, n] = Y_shard[mo*128 + p, n]
"""

import numpy as np
import ml_dtypes

import concourse.bass as bass
import concourse.mybir as mybir
import concourse.tile as tile
from concourse import bacc
from concourse.bass_utils import run_bass_kernel_spmd

P = 128
SEQ, BATCH, D_IN, D_OUT = 8192, 4, 1024, 4096
N_CORES = 8
TOK = SEQ * BATCH
TOK_SHARD = TOK // N_CORES     # 4096
KO = D_IN // P                 # 8
KH = KO // 2                   # 4 (ko half for the split lead-in tiles)
M_TILE = 512
M_OUTER = TOK_SHARD // M_TILE  # 8
M_SUB = M_TILE // P            # 4
N_TILE = 512
N_TILES = D_OUT // N_TILE      # 8
WARM_MM = 18                   # full-K warmup bridging the ~8us preamble to the
                               # ~11us arrival of the first contiguous x/w tiles

BF16 = ml_dtypes.bfloat16

_CACHE = {}

# Last BassKernelResults, for test harnesses that want exec_time_ns.
LAST_RESULT = None


def _build():
    if "nc" in _CACHE:
        return _CACHE["nc"], _CACHE["names"]

    nc = bacc.Bacc(None, target_bir_lowering=False, debug=False)
    with tile.TileContext(nc) as tc:
        with (
            tc.tile_pool(name="dram", bufs=1, space="DRAM") as dram,
            tc.tile_pool(name="consts", bufs=1) as consts,
            tc.tile_pool(name="xpool", bufs=2) as xpool,
            tc.tile_pool(name="opool", bufs=4) as opool,
            tc.tile_pool(name="pspool", bufs=8, space="PSUM") as pspool,
        ):
            xT = dram.tile((M_OUTER, P, KO, M_TILE), mybir.dt.bfloat16, kind="ExternalInput")
            wT = dram.tile((N_TILES, P, KO, N_TILE), mybir.dt.bfloat16, kind="ExternalInput")
            bias_d = dram.tile((D_OUT,), mybir.dt.float32r, kind="ExternalInput")
            out = dram.tile(
                (P, TOK_SHARD // P, D_OUT), mybir.dt.bfloat16, kind="ExternalOutput"
            )

            # Full-K warmup source: no DMA dependency, so the PE starts within
            # ~1us of engine bring-up and HAM reaches K=8/8 before real work.
            warm = consts.tile([P, N_TILE], mybir.dt.float32r, name="warm")
            nc.vector.memset(warm[:].bitcast(mybir.dt.float32), 0.0)
            warm_ps = pspool.tile([P, N_TILE], mybir.dt.float32, name="ps")
            for _ in range(WARM_MM):
                nc.tensor.matmul(
                    warm_ps[:], warm[:, :P], warm[:], start=True, stop=True
                )

            # bias arrives as one 16KB row; K=1 matmul against a ones row
            # broadcasts it to all 128 partitions without a 2.1MB DMA.
            ones = consts.tile([1, P], mybir.dt.float32r, name="ones")
            nc.vector.memset(ones[:].bitcast(mybir.dt.float32), 1.0)
            bias_row = consts.tile([1, D_OUT], mybir.dt.float32r, name="bias_row")
            bias_sb = consts.tile([P, D_OUT], mybir.dt.float32, name="bias_sb")
            bias_1p = bass.AP(
                tensor=bias_d.tensor,
                offset=bias_d.offset,
                ap=[[0, 1], *bias_d.ap],
            )
            nc.sync.dma_start(out=bias_row[:], in_=bias_1p)
            for n in range(N_TILES):
                bps = pspool.tile([P, N_TILE], mybir.dt.float32, name="ps")
                nc.tensor.matmul(
                    bps[:],
                    ones[:],
                    bias_row[:, n * N_TILE : (n + 1) * N_TILE],
                    start=True,
                    stop=True,
                )
                nc.vector.tensor_copy(
                    bias_sb[:, n * N_TILE : (n + 1) * N_TILE], bps[:]
                )

            def load_xm(mo):
                # Two half-tiles per m-tile; the tile-major DRAM layout makes
                # each half one 4KB-contiguous line per partition (the old
                # [P,KO,TOK] layout produced 1KB lines and the DMA queues are
                # line-rate bound at ~57 lines/us during the prologue).
                a = xpool.tile([P, KH, M_TILE], mybir.dt.bfloat16, name="xma")
                b = xpool.tile([P, KH, M_TILE], mybir.dt.bfloat16, name="xmb")
                nc.scalar.dma_start(out=a[:], in_=xT[mo, :, :KH, :])
                nc.scalar.dma_start(out=b[:], in_=xT[mo, :, KH:, :])
                return (a, b)

            def xm_slice(xm_pair, ko, mi):
                t = xm_pair[0] if ko < KH else xm_pair[1]
                k = ko if ko < KH else ko - KH
                return t[:, k : k + 1, mi * P : (mi + 1) * P]

            # The input DMAs drain one HW queue serially at HBM rate, so
            # emission order == arrival order. First m-tile of X goes first,
            # then the W columns in consumption order: the first matmul group
            # needs only the first halves of xm0 + w_col0 (2MB), not 32MB.
            xm_next = load_xm(0)
            w_cols = []
            for n in range(N_TILES):
                if n == 0:
                    wa = consts.tile([P, KH, N_TILE], mybir.dt.bfloat16, name="w0a")
                    wb = consts.tile([P, KH, N_TILE], mybir.dt.bfloat16, name="w0b")
                    nc.sync.dma_start(out=wa[:], in_=wT[0, :, :KH, :])
                    nc.sync.dma_start(out=wb[:], in_=wT[0, :, KH:, :])
                    w_cols.append((wa, wb))
                else:
                    wc = consts.tile(
                        [P, KO, N_TILE], mybir.dt.bfloat16, name=f"w_{n}"
                    )
                    # odd cols ride the otherwise-idle gpsimd SWDGE ring so
                    # each W queue only has to sustain ~140GB/s in m-tile 0
                    w_eng = nc.gpsimd if n % 2 else nc.sync
                    w_eng.dma_start(out=wc[:], in_=wT[n])
                    w_cols.append((wc, wc))

            def w_slice(n, ko):
                a, b = w_cols[n]
                if a is b:
                    return a[:, ko, :]
                t = a if ko < KH else b
                k = ko if ko < KH else ko - KH
                return t[:, k, :]

            for mo in range(M_OUTER):
                xm = xm_next
                if mo + 1 < M_OUTER:
                    xm_next = load_xm(mo + 1)
                # n outer: consumption order matches the W column DMA arrival
                # order, so the first m-tile overlaps the weight prologue
                for n in range(N_TILES):
                    for mi in range(M_SUB):
                        ps = pspool.tile([P, N_TILE], mybir.dt.float32, name="ps")
                        for ko in range(KO):
                            nc.tensor.matmul(
                                ps[:],
                                xm_slice(xm, ko, mi),
                                w_slice(n, ko),
                                start=(ko == 0),
                                stop=(ko == KO - 1),
                            )
                        ot = opool.tile([P, N_TILE], mybir.dt.bfloat16, name="ot")
                        nc.vector.tensor_add(
                            ot[:], ps[:], bias_sb[:, n * N_TILE : (n + 1) * N_TILE]
                        )
                        # outputs alternate rings by m-tile to balance the
                        # 34 MB of writes without queuing ahead of input loads
                        out_eng = nc.sync if mo % 2 else nc.scalar
                        out_eng.dma_start(
                            out=out[:, mo * M_SUB + mi, n * N_TILE : (n + 1) * N_TILE],
                            in_=ot[:],
                        )
    nc.finalize()

    names = (xT.name, wT.name, bias_d.name, out.name)
    _CACHE["nc"] = nc
    _CACHE["names"] = names
    return nc, names


def kernel(x: np.ndarray, weight: np.ndarray, bias: np.ndarray) -> np.ndarray:
    global LAST_RESULT
    nc, (xT_name, wT_name, bias_name, out_name) = _build()

    x = np.ascontiguousarray(x, dtype=np.float32)
    weight = np.ascontiguousarray(weight, dtype=np.float32)
    bias = np.ascontiguousarray(bias, dtype=np.float32)

    # [core, p, ko, m] with x[tok, k] -> xT[p, ko, m] = X_shard[m, ko*128+p]
    # [core, mo, p, ko, mm]: each m-tile is a contiguous [P, KO, M_TILE]
    # block so its DMA runs at 8KB-per-partition line granularity
    xT_all = np.ascontiguousarray(
        x.reshape(N_CORES, M_OUTER, M_TILE, KO, P)
        .transpose(0, 1, 4, 3, 2)
        .astype(BF16)
    )
    # [n, p, ko, nn]: each W column tile is a contiguous block
    wT_dev = np.ascontiguousarray(
        weight.reshape(N_TILES, N_TILE, KO, P).transpose(0, 3, 2, 1).astype(BF16)
    )

    in_maps = [
        {xT_name: xT_all[c], wT_name: wT_dev, bias_name: bias}
        for c in range(N_CORES)
    ]
    res = run_bass_kernel_spmd(nc, in_maps, list(range(N_CORES)))
    LAST_RESULT = res

    # out[p, mo, n] -> Y_shard[mo*128+p, n]; stack shards along tokens
    y = np.empty((TOK, D_OUT), dtype=np.float32)
    for c in range(N_CORES):
        o = np.asarray(res.results[c][out_name], dtype=np.float32)  # [128, 32, 4096]
        y[c * TOK_SHARD : (c + 1) * TOK_SHARD] = o.transpose(1, 0, 2).reshape(
            TOK_SHARD, D_OUT
        )
    return y.reshape(SEQ, BATCH, D_OUT)


# revision 17
# speedup vs baseline: 1.0046x; 1.0046x over previous
"""Column-parallel linear Y = X @ W^T + b on 8 Trainium2 NeuronCores.

Strategy: sequence-shard X across the 8 cores (4096 tokens each); every core
holds the full weight, computes its token slab against all 4096 output
features, so no collective is needed and no core re-reads another's tokens.

Final version (476us vs 511us staged baseline; PE-stream roofline 437us):
  * bf16 matmul operands at the PE's 1 col/cycle: measured 216ns issue gap
    per N=512 matmul (the NX floor) vs 227ns for fp32r, whose per-matmul
    4-byte self-loading weight path leaks ~11ns into the gap. bf16 also
    halves input HBM traffic, which removes the m-tile-0 weight-starvation
    stalls. (Note: chip-wide 2.4->2.0GHz downclock episodes hit runs at
    random and masquerade as dtype effects; compare spacing, not totals.)
  * 40 full-K warmup matmuls on a memset tile run under the ~7us framework
    preamble + DMA lead-in so HAM is at K=8/8 when real work starts
    (K=1 matmuls do NOT register as PE activity for HAM).
  * bias rides in as a 16KB row and is broadcast on-chip with K=1 matmuls
    against a memset ones row (a 2.1MB DMA broadcast would fight the
    prologue for HBM bandwidth). Keep it ONE DMA: chunked bias loads from a
    small pool head-of-line block the sync queue behind PE progress.
  * xm0/w_col0 are ko-split in half so the first accumulation group only
    waits on ~1MB; W columns alternate between the sync and gpsimd rings so
    each ring sustains ~140GB/s during m-tile 0; the next m-tile prefetch
    is deferred to mid-sweep.
  * outputs are written bf16 and upcast on host: halves the 64MB of writes
    that compete with the W stream early on. Total norm rel err ~2.6e-3
    vs the 2e-2 gate.

Device layout (per core):
  xT   [128, 8, 4096]  bf16   xT[p, ko, m] = X_shard[m, ko*128 + p]
  wT   [128, 8, 4096]  bf16   wT[p, ko, n] = W[n, ko*128 + p]
  bias [4096]          fp32   (fp32r bits for the K=1 broadcast matmuls)
  out  [128, 32, 4096] bf16   out[p, mo, n] = Y_shard[mo*128 + p, n]
"""

import numpy as np
import ml_dtypes

import concourse.bass as bass
import concourse.mybir as mybir
import concourse.tile as tile
from concourse import bacc
from concourse.bass_utils import run_bass_kernel_spmd

P = 128
SEQ, BATCH, D_IN, D_OUT = 8192, 4, 1024, 4096
N_CORES = 8
TOK = SEQ * BATCH
TOK_SHARD = TOK // N_CORES     # 4096
KO = D_IN // P                 # 8
KH = KO // 2                   # 4 (ko half for the split lead-in tiles)
M_TILE = 512
M_OUTER = TOK_SHARD // M_TILE  # 8
M_SUB = M_TILE // P            # 4
N_TILE = 512
N_TILES = D_OUT // N_TILE      # 8
WARM_MM = 18                   # full-K warmup bridging the ~8us preamble to the
                               # ~11us arrival of the first contiguous x/w tiles

BF16 = ml_dtypes.bfloat16

_CACHE = {}

# Last BassKernelResults, for test harnesses that want exec_time_ns.
LAST_RESULT = None


def _build():
    if "nc" in _CACHE:
        return _CACHE["nc"], _CACHE["names"]

    nc = bacc.Bacc(None, target_bir_lowering=False, debug=False)
    with tile.TileContext(nc) as tc:
        with (
            tc.tile_pool(name="dram", bufs=1, space="DRAM") as dram,
            tc.tile_pool(name="consts", bufs=1) as consts,
            tc.tile_pool(name="xpool", bufs=2) as xpool,
            tc.tile_pool(name="opool", bufs=4) as opool,
            tc.tile_pool(name="pspool", bufs=8, space="PSUM") as pspool,
        ):
            xT = dram.tile((M_OUTER, P, KO, M_TILE), mybir.dt.bfloat16, kind="ExternalInput")
            wT = dram.tile((N_TILES, P, KO, N_TILE), mybir.dt.bfloat16, kind="ExternalInput")
            bias_d = dram.tile((D_OUT,), mybir.dt.float32r, kind="ExternalInput")
            out = dram.tile(
                (P, TOK_SHARD // P, D_OUT), mybir.dt.bfloat16, kind="ExternalOutput"
            )

            # Full-K warmup source: no DMA dependency, so the PE starts within
            # ~1us of engine bring-up and HAM reaches K=8/8 before real work.
            warm = consts.tile([P, N_TILE], mybir.dt.float32r, name="warm")
            nc.vector.memset(warm[:].bitcast(mybir.dt.float32), 0.0)
            warm_ps = pspool.tile([P, N_TILE], mybir.dt.float32, name="ps")
            for _ in range(WARM_MM):
                nc.tensor.matmul(
                    warm_ps[:], warm[:, :P], warm[:], start=True, stop=True
                )

            # bias arrives as one 16KB row; K=1 matmul against a ones row
            # broadcasts it to all 128 partitions without a 2.1MB DMA.
            ones = consts.tile([1, P], mybir.dt.float32r, name="ones")
            nc.vector.memset(ones[:].bitcast(mybir.dt.float32), 1.0)
            bias_row = consts.tile([1, D_OUT], mybir.dt.float32r, name="bias_row")
            bias_sb = consts.tile([P, D_OUT], mybir.dt.float32, name="bias_sb")
            bias_1p = bass.AP(
                tensor=bias_d.tensor,
                offset=bias_d.offset,
                ap=[[0, 1], *bias_d.ap],
            )
            nc.sync.dma_start(out=bias_row[:], in_=bias_1p)
            for n in range(N_TILES):
                bps = pspool.tile([P, N_TILE], mybir.dt.float32, name="ps")
                nc.tensor.matmul(
                    bps[:],
                    ones[:],
                    bias_row[:, n * N_TILE : (n + 1) * N_TILE],
                    start=True,
                    stop=True,
                )
                nc.vector.tensor_copy(
                    bias_sb[:, n * N_TILE : (n + 1) * N_TILE], bps[:]
                )

            def xm_dram_ap(mo, half):
                # One 4KB row per partition: the DMA queues process ~60
                # descriptor rows/us regardless of row size, so a 3D AP that
                # splits each partition into KO 1KB rows runs 4-8x slower
                # than the same bytes as a single row.
                return bass.AP(
                    tensor=xT.tensor,
                    offset=xT.offset
                    + mo * P * KO * M_TILE
                    + half * KH * M_TILE,
                    ap=[[KO * M_TILE, P], [1, KH * M_TILE]],
                )

            def load_xm(mo):
                a = xpool.tile([P, KH * M_TILE], mybir.dt.bfloat16, name="xma")
                b = xpool.tile([P, KH * M_TILE], mybir.dt.bfloat16, name="xmb")
                nc.scalar.dma_start(out=a[:], in_=xm_dram_ap(mo, 0))
                nc.scalar.dma_start(out=b[:], in_=xm_dram_ap(mo, 1))
                return (a, b)

            def xm_slice(xm_pair, ko, mi):
                t = xm_pair[0] if ko < KH else xm_pair[1]
                k = ko if ko < KH else ko - KH
                base = k * M_TILE + mi * P
                return t[:, base : base + P]

            # The input DMAs drain one HW queue serially at HBM rate, so
            # emission order == arrival order. First m-tile of X goes first,
            # then the W columns in consumption order: the first matmul group
            # needs only the first halves of xm0 + w_col0 (2MB), not 32MB.
            xm_next = load_xm(0)
            w_cols = []
            for n in range(N_TILES):
                def w_dram_ap(n, half, rows):
                    return bass.AP(
                        tensor=wT.tensor,
                        offset=wT.offset
                        + n * P * KO * N_TILE
                        + half * KH * N_TILE,
                        ap=[[KO * N_TILE, P], [1, rows * N_TILE]],
                    )

                if n == 0:
                    wa = consts.tile([P, KH * N_TILE], mybir.dt.bfloat16, name="w0a")
                    wb = consts.tile([P, KH * N_TILE], mybir.dt.bfloat16, name="w0b")
                    nc.sync.dma_start(out=wa[:], in_=w_dram_ap(0, 0, KH))
                    nc.sync.dma_start(out=wb[:], in_=w_dram_ap(0, 1, KH))
                    w_cols.append((wa, wb))
                else:
                    wc = consts.tile(
                        [P, KO * N_TILE], mybir.dt.bfloat16, name=f"w_{n}"
                    )
                    # odd cols ride the otherwise-idle gpsimd SWDGE ring so
                    # each W queue only has to sustain ~140GB/s in m-tile 0
                    w_eng = nc.gpsimd if n % 2 else nc.sync
                    w_eng.dma_start(out=wc[:], in_=w_dram_ap(n, 0, KO))
                    w_cols.append((wc, wc))

            def w_slice(n, ko):
                a, b = w_cols[n]
                if a is b:
                    return a[:, ko * N_TILE : (ko + 1) * N_TILE]
                t = a if ko < KH else b
                k = ko if ko < KH else ko - KH
                return t[:, k * N_TILE : (k + 1) * N_TILE]

            for mo in range(M_OUTER):
                xm = xm_next
                if mo + 1 < M_OUTER:
                    xm_next = load_xm(mo + 1)
                # n outer: consumption order matches the W column DMA arrival
                # order, so the first m-tile overlaps the weight prologue
                for n in range(N_TILES):
                    for mi in range(M_SUB):
                        ps = pspool.tile([P, N_TILE], mybir.dt.float32, name="ps")
                        for ko in range(KO):
                            nc.tensor.matmul(
                                ps[:],
                                xm_slice(xm, ko, mi),
                                w_slice(n, ko),
                                start=(ko == 0),
                                stop=(ko == KO - 1),
                            )
                        ot = opool.tile([P, N_TILE], mybir.dt.bfloat16, name="ot")
                        nc.vector.tensor_add(
                            ot[:], ps[:], bias_sb[:, n * N_TILE : (n + 1) * N_TILE]
                        )
                        # outputs alternate rings per group: all of one
                        # m-tile's outputs on a single ring is 74 rows/us,
                        # over the ~57 rows/us descriptor cap
                        out_eng = nc.sync if (n * M_SUB + mi) % 2 else nc.scalar
                        out_eng.dma_start(
                            out=out[:, mo * M_SUB + mi, n * N_TILE : (n + 1) * N_TILE],
                            in_=ot[:],
                        )
    nc.finalize()

    names = (xT.name, wT.name, bias_d.name, out.name)
    _CACHE["nc"] = nc
    _CACHE["names"] = names
    return nc, names


def kernel(x: np.ndarray, weight: np.ndarray, bias: np.ndarray) -> np.ndarray:
    global LAST_RESULT
    nc, (xT_name, wT_name, bias_name, out_name) = _build()

    x = np.ascontiguousarray(x, dtype=np.float32)
    weight = np.ascontiguousarray(weight, dtype=np.float32)
    bias = np.ascontiguousarray(bias, dtype=np.float32)

    # [core, p, ko, m] with x[tok, k] -> xT[p, ko, m] = X_shard[m, ko*128+p]
    # [core, mo, p, ko, mm]: each m-tile is a contiguous [P, KO, M_TILE]
    # block so its DMA runs at 8KB-per-partition line granularity
    xT_all = np.ascontiguousarray(
        x.reshape(N_CORES, M_OUTER, M_TILE, KO, P)
        .transpose(0, 1, 4, 3, 2)
        .astype(BF16)
    )
    # [n, p, ko, nn]: each W column tile is a contiguous block
    wT_dev = np.ascontiguousarray(
        weight.reshape(N_TILES, N_TILE, KO, P).transpose(0, 3, 2, 1).astype(BF16)
    )

    in_maps = [
        {xT_name: xT_all[c], wT_name: wT_dev, bias_name: bias}
        for c in range(N_CORES)
    ]
    res = run_bass_kernel_spmd(nc, in_maps, list(range(N_CORES)))
    LAST_RESULT = res

    # out[p, mo, n] -> Y_shard[mo*128+p, n]; stack shards along tokens
    y = np.empty((TOK, D_OUT), dtype=np.float32)
    for c in range(N_CORES):
        o = np.asarray(res.results[c][out_name], dtype=np.float32)  # [128, 32, 4096]
        y[c * TOK_SHARD : (c + 1) * TOK_SHARD] = o.transpose(1, 0, 2).reshape(
            TOK_SHARD, D_OUT
        )
    return y.reshape(SEQ, BATCH, D_OUT)


# revision 19
# speedup vs baseline: 1.0060x; 1.0014x over previous
"""Column-parallel linear Y = X @ W^T + b on 8 Trainium2 NeuronCores.

Strategy: sequence-shard X across the 8 cores (4096 tokens each); every core
holds the full weight, computes its token slab against all 4096 output
features, so no collective is needed and no core re-reads another's tokens.

Final version (476us vs 511us staged baseline; PE-stream roofline 437us):
  * bf16 matmul operands at the PE's 1 col/cycle: measured 216ns issue gap
    per N=512 matmul (the NX floor) vs 227ns for fp32r, whose per-matmul
    4-byte self-loading weight path leaks ~11ns into the gap. bf16 also
    halves input HBM traffic, which removes the m-tile-0 weight-starvation
    stalls. (Note: chip-wide 2.4->2.0GHz downclock episodes hit runs at
    random and masquerade as dtype effects; compare spacing, not totals.)
  * 40 full-K warmup matmuls on a memset tile run under the ~7us framework
    preamble + DMA lead-in so HAM is at K=8/8 when real work starts
    (K=1 matmuls do NOT register as PE activity for HAM).
  * bias rides in as a 16KB row and is broadcast on-chip with K=1 matmuls
    against a memset ones row (a 2.1MB DMA broadcast would fight the
    prologue for HBM bandwidth). Keep it ONE DMA: chunked bias loads from a
    small pool head-of-line block the sync queue behind PE progress.
  * xm0/w_col0 are ko-split in half so the first accumulation group only
    waits on ~1MB; W columns alternate between the sync and gpsimd rings so
    each ring sustains ~140GB/s during m-tile 0; the next m-tile prefetch
    is deferred to mid-sweep.
  * outputs are written bf16 and upcast on host: halves the 64MB of writes
    that compete with the W stream early on. Total norm rel err ~2.6e-3
    vs the 2e-2 gate.

Device layout (per core):
  xT   [128, 8, 4096]  bf16   xT[p, ko, m] = X_shard[m, ko*128 + p]
  wT   [128, 8, 4096]  bf16   wT[p, ko, n] = W[n, ko*128 + p]
  bias [4096]          fp32   (fp32r bits for the K=1 broadcast matmuls)
  out  [128, 32, 4096] bf16   out[p, mo, n] = Y_shard[mo*128 + p, n]
"""

import numpy as np
import ml_dtypes

import concourse.bass as bass
import concourse.mybir as mybir
import concourse.tile as tile
from concourse import bacc
from concourse.bass_utils import run_bass_kernel_spmd

P = 128
SEQ, BATCH, D_IN, D_OUT = 8192, 4, 1024, 4096
N_CORES = 8
TOK = SEQ * BATCH
TOK_SHARD = TOK // N_CORES     # 4096
KO = D_IN // P                 # 8
KH = KO // 2                   # 4 (ko half for the split lead-in tiles)
M_TILE = 512
M_OUTER = TOK_SHARD // M_TILE  # 8
M_SUB = M_TILE // P            # 4
N_TILE = 512
N_TILES = D_OUT // N_TILE      # 8
WARM_MM = 22                   # full-K warmup bridging the ~8us preamble to the
                               # ~15us arrival of the first full x/w tiles

BF16 = ml_dtypes.bfloat16

_CACHE = {}

# Last BassKernelResults, for test harnesses that want exec_time_ns.
LAST_RESULT = None


def _build():
    if "nc" in _CACHE:
        return _CACHE["nc"], _CACHE["names"]

    nc = bacc.Bacc(None, target_bir_lowering=False, debug=False)
    with tile.TileContext(nc) as tc:
        with (
            tc.tile_pool(name="dram", bufs=1, space="DRAM") as dram,
            tc.tile_pool(name="consts", bufs=1) as consts,
            tc.tile_pool(name="xpool", bufs=2) as xpool,
            tc.tile_pool(name="opool", bufs=4) as opool,
            tc.tile_pool(name="pspool", bufs=8, space="PSUM") as pspool,
        ):
            xT = dram.tile((M_OUTER, P, KO, M_TILE), mybir.dt.bfloat16, kind="ExternalInput")
            wT = dram.tile((N_TILES, P, KO, N_TILE), mybir.dt.bfloat16, kind="ExternalInput")
            bias_d = dram.tile((D_OUT,), mybir.dt.float32r, kind="ExternalInput")
            out = dram.tile(
                (P, TOK_SHARD // P, D_OUT), mybir.dt.bfloat16, kind="ExternalOutput"
            )

            # Full-K warmup source: no DMA dependency, so the PE starts within
            # ~1us of engine bring-up and HAM reaches K=8/8 before real work.
            warm = consts.tile([P, N_TILE], mybir.dt.float32r, name="warm")
            nc.vector.memset(warm[:].bitcast(mybir.dt.float32), 0.0)
            warm_ps = pspool.tile([P, N_TILE], mybir.dt.float32, name="ps")
            for _ in range(WARM_MM):
                nc.tensor.matmul(
                    warm_ps[:], warm[:, :P], warm[:], start=True, stop=True
                )

            def load_xm(mo):
                # One DMA per m-tile with a single 8KB row per partition:
                # queue throughput is packet-rate bound (~20 packets/us in
                # the prologue) and packet size == row size, so fewer,
                # bigger rows move strictly faster.
                t = xpool.tile([P, KO * M_TILE], mybir.dt.bfloat16, name="xm")
                nc.scalar.dma_start(
                    out=t[:],
                    in_=bass.AP(
                        tensor=xT.tensor,
                        offset=xT.offset + mo * P * KO * M_TILE,
                        ap=[[KO * M_TILE, P], [1, KO * M_TILE]],
                    ),
                )
                return t

            def xm_slice(t, ko, mi):
                base = ko * M_TILE + mi * P
                return t[:, base : base + P]

            # bias arrives as one 16KB row; K=1 matmul against a ones row
            # broadcasts it to all 128 partitions without a 2.1MB DMA.
            ones = consts.tile([1, P], mybir.dt.float32r, name="ones")
            nc.vector.memset(ones[:].bitcast(mybir.dt.float32), 1.0)
            bias_row = consts.tile([1, D_OUT], mybir.dt.float32r, name="bias_row")
            bias_sb = consts.tile([P, D_OUT], mybir.dt.float32, name="bias_sb")
            bias_1p = bass.AP(
                tensor=bias_d.tensor,
                offset=bias_d.offset,
                ap=[[0, 1], *bias_d.ap],
            )
            nc.scalar.dma_start(out=bias_row[:], in_=bias_1p)
            for n in range(N_TILES):
                bps = pspool.tile([P, N_TILE], mybir.dt.float32, name="ps")
                nc.tensor.matmul(
                    bps[:],
                    ones[:],
                    bias_row[:, n * N_TILE : (n + 1) * N_TILE],
                    start=True,
                    stop=True,
                )
                nc.vector.tensor_copy(
                    bias_sb[:, n * N_TILE : (n + 1) * N_TILE], bps[:]
                )


            # The input DMAs drain one HW queue serially at HBM rate, so
            # emission order == arrival order. First m-tile of X goes first,
            # then the W columns in consumption order: the first matmul group
            # needs only the first halves of xm0 + w_col0 (2MB), not 32MB.
            xm_next = load_xm(0)
            w_cols = []
            for n in range(N_TILES):
                wc = consts.tile([P, KO * N_TILE], mybir.dt.bfloat16, name=f"w_{n}")
                # odd cols ride the otherwise-idle gpsimd SWDGE ring so
                # each W queue only has to move 4 tiles in m-tile 0
                w_eng = nc.gpsimd if n % 2 else nc.sync
                w_eng.dma_start(
                    out=wc[:],
                    in_=bass.AP(
                        tensor=wT.tensor,
                        offset=wT.offset + n * P * KO * N_TILE,
                        ap=[[KO * N_TILE, P], [1, KO * N_TILE]],
                    ),
                )
                w_cols.append(wc)


            def w_slice(n, ko):
                return w_cols[n][:, ko * N_TILE : (ko + 1) * N_TILE]

            for mo in range(M_OUTER):
                xm = xm_next
                if mo + 1 < M_OUTER:
                    xm_next = load_xm(mo + 1)
                # n outer: consumption order matches the W column DMA arrival
                # order, so the first m-tile overlaps the weight prologue
                for n in range(N_TILES):
                    for mi in range(M_SUB):
                        ps = pspool.tile([P, N_TILE], mybir.dt.float32, name="ps")
                        for ko in range(KO):
                            nc.tensor.matmul(
                                ps[:],
                                xm_slice(xm, ko, mi),
                                w_slice(n, ko),
                                start=(ko == 0),
                                stop=(ko == KO - 1),
                            )
                        ot = opool.tile([P, N_TILE], mybir.dt.bfloat16, name="ot")
                        nc.vector.tensor_add(
                            ot[:], ps[:], bias_sb[:, n * N_TILE : (n + 1) * N_TILE]
                        )
                        # outputs alternate rings per group: all of one
                        # m-tile's outputs on a single ring is 74 rows/us,
                        # over the ~57 rows/us descriptor cap
                        out_eng = nc.sync if (n * M_SUB + mi) % 2 else nc.scalar
                        out_eng.dma_start(
                            out=out[:, mo * M_SUB + mi, n * N_TILE : (n + 1) * N_TILE],
                            in_=ot[:],
                        )
    nc.finalize()

    names = (xT.name, wT.name, bias_d.name, out.name)
    _CACHE["nc"] = nc
    _CACHE["names"] = names
    return nc, names


def kernel(x: np.ndarray, weight: np.ndarray, bias: np.ndarray) -> np.ndarray:
    global LAST_RESULT
    nc, (xT_name, wT_name, bias_name, out_name) = _build()

    x = np.ascontiguousarray(x, dtype=np.float32)
    weight = np.ascontiguousarray(weight, dtype=np.float32)
    bias = np.ascontiguousarray(bias, dtype=np.float32)

    # [core, p, ko, m] with x[tok, k] -> xT[p, ko, m] = X_shard[m, ko*128+p]
    # [core, mo, p, ko, mm]: each m-tile is a contiguous [P, KO, M_TILE]
    # block so its DMA runs at 8KB-per-partition line granularity
    xT_all = np.ascontiguousarray(
        x.reshape(N_CORES, M_OUTER, M_TILE, KO, P)
        .transpose(0, 1, 4, 3, 2)
        .astype(BF16)
    )
    # [n, p, ko, nn]: each W column tile is a contiguous block
    wT_dev = np.ascontiguousarray(
        weight.reshape(N_TILES, N_TILE, KO, P).transpose(0, 3, 2, 1).astype(BF16)
    )

    in_maps = [
        {xT_name: xT_all[c], wT_name: wT_dev, bias_name: bias}
        for c in range(N_CORES)
    ]
    res = run_bass_kernel_spmd(nc, in_maps, list(range(N_CORES)))
    LAST_RESULT = res

    # out[p, mo, n] -> Y_shard[mo*128+p, n]; stack shards along tokens
    y = np.empty((TOK, D_OUT), dtype=np.float32)
    for c in range(N_CORES):
        o = np.asarray(res.results[c][out_name], dtype=np.float32)  # [128, 32, 4096]
        y[c * TOK_SHARD : (c + 1) * TOK_SHARD] = o.transpose(1, 0, 2).reshape(
            TOK_SHARD, D_OUT
        )
    return y.reshape(SEQ, BATCH, D_OUT)
